# revision 10
# baseline (speedup 1.0000x reference)
"""Trainium2 Bass kernel for AdaptiveGraphLearning (retrieval_knn).

Computes, for X [8192,128], A_raw [8192,8192], lambda scalar:
  Xn = X / max(||X||_2, 1e-12)   (row-normalize)
  S  = Xn @ Xn.T                 (cosine similarity)
  A  = dense top-(K+1) per row with self-edge dropped, row-normalized
  A_final = sigmoid(lam)*A_raw + (1-sigmoid(lam))*A_learned
Returns (A_final, A_learned).

Distribution: row-shard N across 8 cores (1024 rows/core). Each core gets
the full Xn ROTATED by its row offset so the SPMD graph is identical on
all cores (self-similarity diagonal of row-tile t falls at local columns
[t*128,(t+1)*128)). A_raw shards are column-rotated the same way; outputs
are un-rotated after the gather.

v4 (bf16 IO, 3-stage pipeline, measured-clock engine balance):
 - Host supplies XnT (normalized-transposed X, f32, 4MB) and A_raw
   pre-scaled by sigmoid(lambda) in bf16. S must be computed in fp32:
   bf16 matmul inputs perturb S by ~2.5e-4, which flips the 10th/11th
   neighbor in ~4% of rows and fails the fro gate at 8.6e-2 (measured).
 - Per row-tile [128, 8192]: PE fp32 matmuls -> ACT drains PSUM->SBUF
   f32 (2048-wide) -> DVE max8 scan (4x2048 -> 32 candidates) -> top-16
   via max8/match_replace/max8 -> tau = 10th value -> DVE select
   SEL=(S>=tau)*S (bf16, accum_out = row-sum for free) ->
   invr=1/(sum+1e-6).  A_learned = SEL*invr on ACT (slack engine);
   A_final = (SEL*(omlam*invr)) + lam*A_raw as ONE DVE stt (bf16 2x
   mode) in place over the prefetched A_raw tile.
 - 3-stage software pipeline: iter i = matmul+drain(i) || scan(i-1) ||
   select+blend+stores(i-2), so every DVE op at iter i depends only on
   work from earlier iterations (no intra-iteration cross-engine waits).
Measured engine rates: fp32 matmul pass 690ns/512col, ACT 1x
dtype-independent, DVE 1x fp32 / 2x bf16-tt / 4x bf16-ts.
Budget/tile: PE 22.1us, DVE ~22.9us, ACT 15.4us, DMA 17.6us.
"""

import numpy as np

N = 8192
D = 128
NCORES = 8
RPC = N // NCORES   # rows per core
P = 128
TILES = RPC // P    # row tiles per core: 8
QW = 2048           # psum group width (4 banks f32)
NQ = N // QW        # groups per row: 4
MMF = 512           # matmul moving free dim (one PSUM bank, f32)
HW = N // 2         # stage-B half width

BLEND = "dve"       # 'dve': stt blend; 'cce': SBUF->SBUF accum-DMA blend

LAST_RESULTS = None
_NC_CACHE = None


def _build():
    import concourse.mybir as mybir
    import concourse.tile as tile
    from concourse import bacc
    from concourse.bass import ts

    f32 = mybir.dt.float32
    bf16 = mybir.dt.bfloat16
    AF = mybir.ActivationFunctionType
    OP = mybir.AluOpType

    nc = bacc.Bacc("TRN2", target_bir_lowering=False, debug=False,
                   num_devices=NCORES)

    XnT_d = nc.dram_tensor("XnT", [P, N], f32, kind="ExternalInput")
    A_d = nc.dram_tensor("A_raw", [RPC, N], bf16, kind="ExternalInput")
    lam_d = nc.dram_tensor("lam", [P, 1], f32, kind="ExternalInput")
    AF_d = nc.dram_tensor("A_final", [RPC, N], bf16, kind="ExternalOutput")
    AL_d = nc.dram_tensor("A_learned", [RPC, N], bf16, kind="ExternalOutput")

    with tile.TileContext(nc) as tc:
        with (
            tc.tile_pool(name="const", bufs=1) as constp,
            tc.tile_pool(name="st", bufs=2) as stp,
            tc.tile_pool(name="sel", bufs=2) as selp,
            tc.tile_pool(name="araw", bufs=2) as arawp,
            tc.tile_pool(name="small", bufs=3) as smallp,
            tc.tile_pool(name="psum", bufs=2, space="PSUM") as psump,
        ):
            # lambda -> sigmoid -> omlam on device ([P,1], replicated)
            lam_sb = constp.tile([P, 1], f32, name="lam_sb")
            nc.sync.dma_start(lam_sb[:], lam_d.ap())
            lam_bc = constp.tile([P, 1], f32, name="lam_bc")
            nc.scalar.activation(lam_bc[:], lam_sb[:], AF.Sigmoid)
            omlam = constp.tile([P, 1], f32, name="omlam")
            nc.scalar.activation(omlam[:], lam_bc[:], AF.Copy, bias=1.0,
                                 scale=-1.0)

            # notI: 1 everywhere except 0 on the diagonal
            notI = constp.tile([P, P], f32, name="notI")
            nc.gpsimd.memset(notI[:], 1.0)
            nc.gpsimd.affine_select(
                out=notI[:], in_=notI[:], pattern=[[-1, P]],
                compare_op=OP.not_equal, fill=0.0, base=0,
                channel_multiplier=1)

            # XnT arrives ready; stream per 2048-col chunk so tile 0's
            # matmuls start after the first chunk lands.
            xnt = constp.tile([P, N], f32, name="xnt")
            for q in range(NQ):
                nc.sync.dma_start(xnt[:, ts(q, QW)], XnT_d.ap()[:, ts(q, QW)])

            s_tiles = {}
            araw_tiles = {}
            sel_tiles = {}
            cands = {}
            g2s = {}

            def fetch_araw(t):
                araw_t = arawp.tile([P, N], bf16, name=f"araw{t}",
                                    tag="araw")
                nc.sync.dma_start(araw_t[:], A_d.ap()[ts(t, P), :])
                araw_tiles[t] = araw_t

            fetch_araw(0)

            def stage_A(t):
                # PE fp32 matmuls + ACT drains, 4 groups of 2048
                s_tiles[t] = stp.tile([P, N], f32, name=f"s{t}", tag="st")
                for q in range(NQ):
                    pm = psump.tile([P, QW], f32, name=f"smm{t}_{q}",
                                    tag="mm")
                    for j in range(QW // MMF):
                        nc.tensor.matmul(
                            pm[:, ts(j, MMF)], xnt[:, ts(t, P)],
                            xnt[:, ts(q * (QW // MMF) + j, MMF)],
                            start=True, stop=True)
                    nc.scalar.activation(s_tiles[t][:, ts(q, QW)], pm[:],
                                         AF.Copy)

            def stage_S(t):
                # DVE: diag zero, chunked max8 scan, top-16 -> tau
                s_t = s_tiles[t]
                nc.vector.tensor_mul(s_t[:, ts(t, P)], s_t[:, ts(t, P)],
                                     notI[:])
                cand = smallp.tile([P, 32], f32, name=f"cand{t}", tag="cand")
                for q in range(NQ):
                    nc.vector.max(cand[:, ts(q, 8)], s_t[:, ts(q, QW)])
                g1 = smallp.tile([P, 8], f32, name=f"g1_{t}", tag="g1")
                nc.vector.max(g1[:], cand[:])
                nc.vector.match_replace(out=cand[:], in_to_replace=g1[:],
                                        in_values=cand[:], imm_value=-1e30)
                g2 = smallp.tile([P, 8], f32, name=f"g2_{t}", tag="g2")
                nc.vector.max(g2[:], cand[:])
                cands[t] = cand
                g2s[t] = g2

            def stage_B1(t):
                # DVE: halved selects (+row-sum), invr chain; ACT: scale
                # sel in place to ALom = SEL*(invr*omlam) (the host
                # divides A_learned by omlam); AL stores per half.
                s_t = s_tiles[t]
                g2 = g2s[t]
                sel_t = selp.tile([P, N], bf16, name=f"sel{t}", tag="sel")
                sel_tiles[t] = sel_t
                rs0 = smallp.tile([P, 1], f32, name=f"rs0_{t}", tag="rs0")
                rs1 = smallp.tile([P, 1], f32, name=f"rs1_{t}", tag="rs1")
                for h, rs in ((0, rs0), (1, rs1)):
                    hs = ts(h, HW)
                    nc.vector.scalar_tensor_tensor(
                        out=sel_t[:, hs], in0=s_t[:, hs], scalar=g2[:, 1:2],
                        in1=s_t[:, hs], op0=OP.is_ge, op1=OP.mult,
                        accum_out=rs[:])
                w2 = smallp.tile([P, 1], f32, name=f"w2_{t}", tag="w2")
                nc.vector.tensor_add(w2[:], rs0[:], rs1[:])
                nc.vector.tensor_scalar_add(w2[:], w2[:], 1e-6)
                nc.vector.reciprocal(w2[:], w2[:])
                nc.vector.tensor_mul(w2[:], w2[:], omlam[:])
                for h in range(2):
                    hs = ts(h, HW)
                    nc.scalar.activation(sel_t[:, hs], sel_t[:, hs],
                                         AF.Copy, scale=w2[:])
                    nc.sync.dma_start(AL_d.ap()[ts(t, P), hs],
                                      sel_t[:, hs])

            def stage_B2(t):
                # DVE: A_final = ALom + lam*A_raw as plain bf16
                # tensor_add (2x mode), in place on the A_raw tile.
                araw_t = araw_tiles[t]
                sel_t = sel_tiles[t]
                for h in range(2):
                    hs = ts(h, HW)
                    nc.vector.tensor_add(araw_t[:, hs], araw_t[:, hs],
                                         sel_t[:, hs])
                    nc.sync.dma_start(AF_d.ap()[ts(t, P), hs],
                                      araw_t[:, hs])

            # 3-stage pipeline: A(i) matmul+drain; [S+B1](i-1) scans then
            # selects; B2(i-2) blend. DVE order inside an iteration is
            # scans(i-1) -> blend(i-2) -> selects(i-1) so every op's deps
            # come from previous iterations or earlier DVE ops.
            for i in range(TILES + 2):
                tA, tS, tB2 = i, i - 1, i - 2
                if 1 <= tA <= TILES - 1:
                    fetch_araw(tA)
                if 0 <= tS < TILES:
                    stage_S(tS)
                if 0 <= tB2 < TILES:
                    stage_B2(tB2)
                if 0 <= tS < TILES:
                    stage_B1(tS)
                    if tS == TILES - 1:
                        stage_B2(tS)
                        break
                if tA < TILES:
                    stage_A(tA)

    nc.compile()
    return nc


def kernel(X, A_raw, lambda_param):
    global LAST_RESULTS, _NC_CACHE
    import ml_dtypes
    from concourse.bass_utils import run_bass_kernel_spmd

    BF16 = np.dtype(ml_dtypes.bfloat16)
    X = np.asarray(X, dtype=np.float32)
    A_raw = np.asarray(A_raw, dtype=np.float32)
    lam_v = float(np.asarray(lambda_param, dtype=np.float32).reshape(()))
    lam = 1.0 / (1.0 + np.exp(-lam_v))
    omlam = 1.0 - lam

    if _NC_CACHE is None:
        _NC_CACHE = _build()
    nc = _NC_CACHE

    norms = np.maximum(np.sqrt((X.astype(np.float64) ** 2).sum(axis=1)),
                       1e-12)
    Xn = (X / norms[:, None].astype(np.float32)).astype(np.float32)

    # pre-scale A_raw by lam ('dve') or lam/omlam ('cce') during the bf16
    # conversion; the learned part keeps its own scaling on device.
    pre = lam if BLEND == "dve" else lam / omlam
    lam_in = np.full((P, 1), lam_v, dtype=np.float32)
    in_maps = []
    for c in range(NCORES):
        r0 = c * RPC
        XnT = np.ascontiguousarray(np.roll(Xn, -r0, axis=0).T)
        in_maps.append({
            "XnT": XnT,
            "A_raw": (np.roll(A_raw[r0:r0 + RPC], -r0, axis=1)
                      * np.float32(pre)).astype(BF16),
            "lam": lam_in,
        })

    res = run_bass_kernel_spmd(nc, in_maps, core_ids=list(range(NCORES)))
    LAST_RESULTS = res

    A_final = np.empty((N, N), dtype=np.float32)
    A_learned = np.empty((N, N), dtype=np.float32)
    for c in range(NCORES):
        r0 = c * RPC
        A_final[r0:r0 + RPC] = np.roll(
            res.results[c]["A_final"], r0, axis=1).astype(np.float32)
        A_learned[r0:r0 + RPC] = np.roll(
            res.results[c]["A_learned"], r0, axis=1).astype(np.float32)
    if BLEND == "cce":
        A_final *= np.float32(omlam)
    A_learned *= np.float32(1.0 / omlam)
    return A_final, A_learned


# revision 13
# speedup vs baseline: 1.1979x; 1.1979x over previous
"""Trainium2 Bass kernel for AdaptiveGraphLearning (retrieval_knn).

Computes, for X [8192,128], A_raw [8192,8192], lambda scalar:
  Xn = X / max(||X||_2, 1e-12)   (row-normalize)
  S  = Xn @ Xn.T                 (cosine similarity)
  A  = dense top-(K+1) per row with self-edge dropped, row-normalized
  A_final = sigmoid(lam)*A_raw + (1-sigmoid(lam))*A_learned
Returns (A_final, A_learned).

Distribution: row-shard N across 8 cores (1024 rows/core). Each core gets
the full Xn ROTATED by its row offset so the SPMD graph is identical on
all cores (self-similarity diagonal of row-tile t falls at local columns
[t*128,(t+1)*128)). A_raw shards are column-rotated the same way; outputs
are un-rotated after the gather.

v4 (bf16 IO, 3-stage pipeline, measured-clock engine balance):
 - Host supplies XnT (normalized-transposed X, f32, 4MB) and A_raw
   pre-scaled by sigmoid(lambda) in bf16. S must be computed in fp32:
   bf16 matmul inputs perturb S by ~2.5e-4, which flips the 10th/11th
   neighbor in ~4% of rows and fails the fro gate at 8.6e-2 (measured).
 - Per row-tile [128, 8192]: PE fp32 matmuls -> ACT drains PSUM->SBUF
   f32 (2048-wide) -> DVE max8 scan (4x2048 -> 32 candidates) -> top-16
   via max8/match_replace/max8 -> tau = 10th value -> DVE select
   SEL=(S>=tau)*S (bf16, accum_out = row-sum for free) ->
   invr=1/(sum+1e-6).  A_learned = SEL*invr on ACT (slack engine);
   A_final = (SEL*(omlam*invr)) + lam*A_raw as ONE DVE stt (bf16 2x
   mode) in place over the prefetched A_raw tile.
 - 3-stage software pipeline: iter i = matmul+drain(i) || scan(i-1) ||
   select+blend+stores(i-2), so every DVE op at iter i depends only on
   work from earlier iterations (no intra-iteration cross-engine waits).
Measured engine rates: fp32 matmul pass 690ns/512col, ACT 1x
dtype-independent, DVE 1x fp32 / 2x bf16-tt / 4x bf16-ts.
Budget/tile: PE 22.1us, DVE ~22.9us, ACT 15.4us, DMA 17.6us.
"""

import numpy as np

N = 8192
D = 128
NCORES = 8
RPC = N // NCORES   # rows per core
P = 128
TILES = RPC // P    # row tiles per core: 8
QW = 2048           # psum group width (4 banks f32)
NQ = N // QW        # groups per row: 4
MMF = 512           # matmul moving free dim (one PSUM bank, f32)
HW = N // 2         # stage-B half width

BLEND = "dve"       # 'dve': stt blend; 'cce': SBUF->SBUF accum-DMA blend

LAST_RESULTS = None
_NC_CACHE = None


def _build():
    import concourse.mybir as mybir
    import concourse.tile as tile
    from concourse import bacc
    from concourse.bass import ts

    f32 = mybir.dt.float32
    bf16 = mybir.dt.bfloat16
    AF = mybir.ActivationFunctionType
    OP = mybir.AluOpType

    nc = bacc.Bacc("TRN2", target_bir_lowering=False, debug=False,
                   num_devices=NCORES)

    XnT_d = nc.dram_tensor("XnT", [P, N], f32, kind="ExternalInput")
    A_d = nc.dram_tensor("A_raw", [RPC, N], bf16, kind="ExternalInput")
    lam_d = nc.dram_tensor("lam", [P, 1], f32, kind="ExternalInput")
    AF_d = nc.dram_tensor("A_final", [RPC, N], bf16, kind="ExternalOutput")
    AL_d = nc.dram_tensor("A_learned", [RPC, N], bf16, kind="ExternalOutput")

    with tile.TileContext(nc) as tc:
        with (
            tc.tile_pool(name="const", bufs=1) as constp,
            tc.tile_pool(name="st", bufs=2) as stp,
            tc.tile_pool(name="sel", bufs=2) as selp,
            tc.tile_pool(name="araw", bufs=2) as arawp,
            tc.tile_pool(name="small", bufs=3) as smallp,
            tc.tile_pool(name="psum", bufs=2, space="PSUM") as psump,
        ):
            # lambda -> sigmoid -> omlam on device ([P,1], replicated)
            lam_sb = constp.tile([P, 1], f32, name="lam_sb")
            nc.sync.dma_start(lam_sb[:], lam_d.ap())
            lam_bc = constp.tile([P, 1], f32, name="lam_bc")
            nc.scalar.activation(lam_bc[:], lam_sb[:], AF.Sigmoid)
            omlam = constp.tile([P, 1], f32, name="omlam")
            nc.scalar.activation(omlam[:], lam_bc[:], AF.Copy, bias=1.0,
                                 scale=-1.0)

            # notI: 1 everywhere except 0 on the diagonal
            notI = constp.tile([P, P], f32, name="notI")
            nc.gpsimd.memset(notI[:], 1.0)
            nc.gpsimd.affine_select(
                out=notI[:], in_=notI[:], pattern=[[-1, P]],
                compare_op=OP.not_equal, fill=0.0, base=0,
                channel_multiplier=1)

            # XnT arrives ready; stream per 2048-col chunk. S is computed
            # as H@H + H@L + L@H with H=bf16(Xn), L=bf16(Xn-H): 3 bf16
            # passes match fp32's 2 slow passes but get fast weight load
            # (fp32 stationaries exclude FWL), and residual L@L ~ 5e-6
            # cannot flip 10th/11th neighbors.
            xnt = constp.tile([P, N], f32, name="xnt")
            xh = constp.tile([P, N], bf16, name="xh")
            xl = constp.tile([P, N], bf16, name="xl")
            for q in range(NQ):
                qs = ts(q, QW)
                nc.sync.dma_start(xnt[:, qs], XnT_d.ap()[:, qs])
                nc.scalar.activation(xh[:, qs], xnt[:, qs], AF.Copy)
                nc.vector.tensor_sub(xl[:, qs], xnt[:, qs], xh[:, qs])

            s_tiles = {}
            araw_tiles = {}
            sel_tiles = {}
            cands = {}
            g2s = {}

            def fetch_araw(t):
                araw_t = arawp.tile([P, N], bf16, name=f"araw{t}",
                                    tag="araw")
                nc.sync.dma_start(araw_t[:], A_d.ap()[ts(t, P), :])
                araw_tiles[t] = araw_t

            fetch_araw(0)

            def stage_A(t):
                # PE fp32 matmuls + ACT drains, 4 groups of 2048
                s_tiles[t] = stp.tile([P, N], f32, name=f"s{t}", tag="st")
                for q in range(NQ):
                    pm = psump.tile([P, QW], f32, name=f"smm{t}_{q}",
                                    tag="mm")
                    for stat, mov, st_f, sp_f in (
                            (xh, xh, True, False),
                            (xh, xl, False, False),
                            (xl, xh, False, True)):
                        for j in range(QW // MMF):
                            nc.tensor.matmul(
                                pm[:, ts(j, MMF)], stat[:, ts(t, P)],
                                mov[:, ts(q * (QW // MMF) + j, MMF)],
                                start=st_f, stop=sp_f)
                    nc.scalar.activation(s_tiles[t][:, ts(q, QW)], pm[:],
                                         AF.Copy)

            def stage_S(t):
                # DVE: diag zero, chunked max8 scan, top-16 -> tau
                s_t = s_tiles[t]
                nc.vector.tensor_mul(s_t[:, ts(t, P)], s_t[:, ts(t, P)],
                                     notI[:])
                cand = smallp.tile([P, 32], f32, name=f"cand{t}", tag="cand")
                for q in range(NQ):
                    nc.vector.max(cand[:, ts(q, 8)], s_t[:, ts(q, QW)])
                g1 = smallp.tile([P, 8], f32, name=f"g1_{t}", tag="g1")
                nc.vector.max(g1[:], cand[:])
                nc.vector.match_replace(out=cand[:], in_to_replace=g1[:],
                                        in_values=cand[:], imm_value=-1e30)
                g2 = smallp.tile([P, 8], f32, name=f"g2_{t}", tag="g2")
                nc.vector.max(g2[:], cand[:])
                cands[t] = cand
                g2s[t] = g2

            def stage_B1(t):
                # DVE: halved selects (+row-sum), invr chain; ACT: scale
                # sel in place to ALom = SEL*(invr*omlam) (the host
                # divides A_learned by omlam); AL stores per half.
                s_t = s_tiles[t]
                g2 = g2s[t]
                sel_t = selp.tile([P, N], bf16, name=f"sel{t}", tag="sel")
                sel_tiles[t] = sel_t
                rs0 = smallp.tile([P, 1], f32, name=f"rs0_{t}", tag="rs0")
                rs1 = smallp.tile([P, 1], f32, name=f"rs1_{t}", tag="rs1")
                for h, rs in ((0, rs0), (1, rs1)):
                    hs = ts(h, HW)
                    nc.vector.scalar_tensor_tensor(
                        out=sel_t[:, hs], in0=s_t[:, hs], scalar=g2[:, 1:2],
                        in1=s_t[:, hs], op0=OP.is_ge, op1=OP.mult,
                        accum_out=rs[:])
                w2 = smallp.tile([P, 1], f32, name=f"w2_{t}", tag="w2")
                nc.vector.tensor_add(w2[:], rs0[:], rs1[:])
                nc.vector.tensor_scalar_add(w2[:], w2[:], 1e-6)
                nc.vector.reciprocal(w2[:], w2[:])
                nc.vector.tensor_mul(w2[:], w2[:], omlam[:])
                for h in range(2):
                    hs = ts(h, HW)
                    nc.scalar.activation(sel_t[:, hs], sel_t[:, hs],
                                         AF.Copy, scale=w2[:])
                    nc.sync.dma_start(AL_d.ap()[ts(t, P), hs],
                                      sel_t[:, hs])

            def stage_B2(t):
                araw_t = araw_tiles[t]
                sel_t = sel_tiles[t]
                if BLEND == "cce":
                    # A_final = ALom + lam*A_raw via SBUF->SBUF CCE-add
                    # (no compute engine); the AL stores in stage_B1 read
                    # sel first (framework WAR ordering, proven pattern).
                    nc.gpsimd.dma_start(sel_t[:], araw_t[:],
                                        accum_op=OP.add)
                    nc.sync.dma_start(AF_d.ap()[ts(t, P), :], sel_t[:])
                    return
                # DVE: A_final = ALom + lam*A_raw as plain bf16
                # tensor_add (2x mode), in place on the A_raw tile.
                for h in range(2):
                    hs = ts(h, HW)
                    nc.vector.tensor_add(araw_t[:, hs], araw_t[:, hs],
                                         sel_t[:, hs])
                    nc.sync.dma_start(AF_d.ap()[ts(t, P), hs],
                                      araw_t[:, hs])

            # 3-stage pipeline: A(i) matmul+drain; [S+B1](i-1) scans then
            # selects; B2(i-2) blend. DVE order inside an iteration is
            # scans(i-1) -> blend(i-2) -> selects(i-1) so every op's deps
            # come from previous iterations or earlier DVE ops.
            for i in range(TILES + 2):
                tA, tS, tB2 = i, i - 1, i - 2
                if 1 <= tA <= TILES - 1:
                    fetch_araw(tA)
                if 0 <= tS < TILES:
                    stage_S(tS)
                if 0 <= tB2 < TILES:
                    stage_B2(tB2)
                if 0 <= tS < TILES:
                    stage_B1(tS)
                    if tS == TILES - 1:
                        stage_B2(tS)
                        break
                if tA < TILES:
                    stage_A(tA)

    nc.compile()
    return nc


def kernel(X, A_raw, lambda_param):
    global LAST_RESULTS, _NC_CACHE
    import ml_dtypes
    from concourse.bass_utils import run_bass_kernel_spmd

    BF16 = np.dtype(ml_dtypes.bfloat16)
    X = np.asarray(X, dtype=np.float32)
    A_raw = np.asarray(A_raw, dtype=np.float32)
    lam_v = float(np.asarray(lambda_param, dtype=np.float32).reshape(()))
    lam = 1.0 / (1.0 + np.exp(-lam_v))
    omlam = 1.0 - lam

    if _NC_CACHE is None:
        _NC_CACHE = _build()
    nc = _NC_CACHE

    norms = np.maximum(np.sqrt((X.astype(np.float64) ** 2).sum(axis=1)),
                       1e-12)
    Xn = (X / norms[:, None].astype(np.float32)).astype(np.float32)

    # pre-scale A_raw by lam ('dve') or lam/omlam ('cce') during the bf16
    # conversion; the learned part keeps its own scaling on device.
    pre = lam if BLEND == "dve" else lam / omlam
    lam_in = np.full((P, 1), lam_v, dtype=np.float32)
    in_maps = []
    for c in range(NCORES):
        r0 = c * RPC
        XnT = np.ascontiguousarray(np.roll(Xn, -r0, axis=0).T)
        in_maps.append({
            "XnT": XnT,
            "A_raw": (np.roll(A_raw[r0:r0 + RPC], -r0, axis=1)
                      * np.float32(pre)).astype(BF16),
            "lam": lam_in,
        })

    res = run_bass_kernel_spmd(nc, in_maps, core_ids=list(range(NCORES)))
    LAST_RESULTS = res

    A_final = np.empty((N, N), dtype=np.float32)
    A_learned = np.empty((N, N), dtype=np.float32)
    for c in range(NCORES):
        r0 = c * RPC
        A_final[r0:r0 + RPC] = np.roll(
            res.results[c]["A_final"], r0, axis=1).astype(np.float32)
        A_learned[r0:r0 + RPC] = np.roll(
            res.results[c]["A_learned"], r0, axis=1).astype(np.float32)
    A_learned *= np.float32(1.0 / omlam)
    return A_final, A_learned
